# revision 6
# baseline (speedup 1.0000x reference)
"""Trainium2 Bass kernel for nn_DiffPoolEncoder (64 graphs x 64 nodes, D=256).

Data-parallel over graphs: 8 graphs per NeuronCore, packed as 4 pairs of two
64-node graphs per 128-partition tile. All matmul operands are bf16 (PSUM
accumulates fp32).

Math reformulations (validated against the jax reference in numpy):
  * Level-2 pooling branch is dead: softmax rows sum to 1, so
    mean_s(S2^T Z2) = colsum(Z2)/P2; colsum(Z2) = H2^T v with v = A1hat^T 1.
  * (1/8) and emb1_W2 @ lin1_W fold into one host-side constant Wfold.
  * Pool branch projects 256->32 BEFORE the second propagation:
    S_logits = Ahat (Hp W2p) instead of (Ahat Hp) W2p.
  * A1hat^T = S^T A^T S + I computed without a raw-A input:
    Q = Ahat S, R = Q - S = A S, A1t = R^T S; the +I is accumulated in PSUM
    via an identity matmul, so the row-sum v needs no +1 fixup.
  * Z = (Ahat He) W2e is built off the critical path via the transposed
    propagation T2t = He^T Ahat^T, so X1 = S^T Z is a single matmul stage
    after the softmax.
  * Softmax skips the max-subtraction (logits are bounded ~ +-55 on this
    distribution; exp stays far below f32/bf16 overflow).
  * lin1/lin2 biases fold into the matmul accumulations as rank-1 updates
    (bias row stationary x ones moving), so each linear needs just one
    PSUM->SBUF eviction and the final result is DMA'd straight from PSUM.
"""
import numpy as np

NC_COUNT = 8
B, NPG, D = 64, 64, 256
GPC = B // NC_COUNT     # graphs per core = 8
PAIRS = GPC // 2        # 4 pair-tiles of 128 nodes per core

# --- consolidated input column layouts (bf16 cols per partition) ---
# dA / dC: [x(256) | ahatT(128)] x 2 pairs             -> 768 cols
# dB1: c*512 + [W1p 0:256 | W1e 256:512]               -> 1024 cols
# dB2: c*288 + [w2p 0:32 | w2e 32:288]                 -> 576 cols
# dD: w1e2 c*256 (512), wfold 512 + c*512 + m*128 (1024),
#     lin2 1536 + c*256 + m*128 (1024),
#     bias rows: row 0, cols 2560+128*k for k in 0..3 -> b1 chunks,
#                row 0, cols 3072+128*k for k in 0..1 -> b2 chunks
DA_COLS = 768
DB1_COLS = 1024
DB2_COLS = 576
DC_COLS = 768
DD_COLS = 3328

_BUILT = None


def _build():
    import concourse.bacc as bacc
    import concourse.tile as tile
    from concourse import mybir

    f32 = mybir.dt.float32
    bf16 = mybir.dt.bfloat16
    nc = bacc.Bacc("TRN2", target_bir_lowering=False, debug=False,
                   num_devices=NC_COUNT)

    dA = nc.dram_tensor("dA", [128, DA_COLS], bf16, kind="ExternalInput")
    dB1 = nc.dram_tensor("dB1", [128, DB1_COLS], bf16, kind="ExternalInput")
    dB2 = nc.dram_tensor("dB2", [128, DB2_COLS], bf16, kind="ExternalInput")
    dC = nc.dram_tensor("dC", [128, DC_COLS], bf16, kind="ExternalInput")
    dD = nc.dram_tensor("dD", [128, DD_COLS], bf16, kind="ExternalInput")
    out_d = nc.dram_tensor("out_t", [128, 16], f32, kind="ExternalOutput")

    Relu = mybir.ActivationFunctionType.Relu
    Exp = mybir.ActivationFunctionType.Exp
    AX = mybir.AxisListType.X

    with tile.TileContext(nc) as tc:
        with (
            tc.tile_pool(name="singles", bufs=1) as sg,
            tc.tile_pool(name="ps_t1", bufs=2, space="PSUM") as ps_t1,
            tc.tile_pool(name="ps_hh", bufs=2, space="PSUM") as ps_hh,
            tc.tile_pool(name="ps_mid", bufs=2, space="PSUM") as ps_mid,
            tc.tile_pool(name="ps_pl", bufs=2, space="PSUM") as ps_pl,
        ):
            # ---------- input DMAs (issued back to back; HWDGE pipelines) ---
            dA_sb = sg.tile([128, DA_COLS], bf16, tag="dA", name="dA_sb")
            nc.sync.dma_start(out=dA_sb, in_=dA[:, :])
            dB1_sb = sg.tile([128, DB1_COLS], bf16, tag="dB1", name="dB1_sb")
            nc.sync.dma_start(out=dB1_sb, in_=dB1[:, :])
            dC_sb = sg.tile([128, DC_COLS], bf16, tag="dC", name="dC_sb")
            nc.sync.dma_start(out=dC_sb, in_=dC[:, :])
            dB2_sb = sg.tile([128, DB2_COLS], bf16, tag="dB2", name="dB2_sb")
            nc.sync.dma_start(out=dB2_sb, in_=dB2[:, :])
            dD_sb = sg.tile([128, DD_COLS], bf16, tag="dD", name="dD_sb")
            nc.sync.dma_start(out=dD_sb, in_=dD[:, :])

            def x_ap(p):
                t = dA_sb if p < 2 else dC_sb
                o = 384 * (p % 2)
                return t[:, o:o + 256]

            def aT_ap(p):
                t = dA_sb if p < 2 else dC_sb
                o = 384 * (p % 2) + 256
                return t[:, o:o + 128]

            def w1p(c, o):
                return dB1_sb[:, 512 * c + 128 * o:512 * c + 128 * (o + 1)]

            def w1e(c):
                return dB1_sb[:, 512 * c + 256:512 * c + 512]

            def w2p(c):
                return dB2_sb[:, 288 * c:288 * c + 32]

            def w2e(c):
                return dB2_sb[:, 288 * c + 32:288 * c + 288]

            def w1e2(c):
                return dD_sb[:, 256 * c:256 * (c + 1)]

            def wfold(c, m):
                o = 512 + 512 * c + 128 * m
                return dD_sb[:, o:o + 128]

            def lin2(c, m):
                o = 1536 + 256 * c + 128 * m
                return dD_sb[:, o:o + 128]

            def b1row(m):
                return dD_sb[0:1, 2560 + 128 * m:2560 + 128 * (m + 1)]

            def b2row(m):
                return dD_sb[0:1, 3072 + 128 * m:3072 + 128 * (m + 1)]

            # ---------- persistent SBUF tiles ------------------------------
            def S(shape, dt, nm):
                return sg.tile(shape, dt, tag=nm, name=nm)

            # trigger the Act engine's activation-table load at t=0 (its
            # 1.28us LoadActFuncSet otherwise lands behind the first
            # eviction's semaphore wait, stalling the whole Act queue)
            ones8 = S([1, 8], bf16, "ones8")
            nc.gpsimd.memset(ones8, 1.0)
            warm = S([1, 8], f32, "warm")
            nc.scalar.activation(out=warm, in_=ones8, func=Relu)

            ident = S([128, 64], f32, "ident")     # I64 stacked twice
            nc.gpsimd.memset(ident, 0.0)
            for h in (0, 1):
                nc.gpsimd.affine_select(
                    out=ident[64 * h:64 * h + 64, :],
                    in_=ident[64 * h:64 * h + 64, :],
                    compare_op=mybir.AluOpType.not_equal,
                    fill=1.0, base=0, pattern=[[-1, 64]], channel_multiplier=1)
            identb = S([128, 64], bf16, "identb")
            nc.vector.tensor_copy(out=identb, in_=ident)

            t1t_sb = [S([128, 512], bf16, f"t1t{g}") for g in range(2)]
            h_sb = [S([128, 512], bf16, f"h{p}") for p in range(PAIRS)]
            g_sb = [S([128, 64], bf16, f"g{g}") for g in range(2)]
            s_exp = [S([128, 2, 32], bf16, f"sexp{g}") for g in range(2)]
            sumexp = [S([128, 2], f32, f"sumexp{g}") for g in range(2)]
            rsum = [S([128, 2], f32, f"rsum{g}") for g in range(2)]
            s_bd = [S([128, 128], bf16, f"sbd{g}") for g in range(2)]
            t2t_sb = [S([128, 512], bf16, f"t2t{g}") for g in range(2)]
            z_sb = [S([128, 512], bf16, f"z{g}") for g in range(2)]
            x1_sb = [S([128, 256], bf16, f"x1{g}") for g in range(2)]
            r_sb = [S([128, 128], bf16, f"r{g}") for g in range(2)]
            a1t_bd = [S([128, 128], bf16, f"a1t{g}") for g in range(2)]
            vred = [S([128, 1], f32, f"vred{g}") for g in range(2)]
            v_bd = [S([128, 8], bf16, f"vbd{g}") for g in range(2)]
            tl2_sb = [S([128, 2, 128], bf16, f"tl2_{g}") for g in range(2)]
            h2_sb = [S([128, 256], bf16, f"h2_{g}") for g in range(2)]
            ut_sb = S([128, 2, 8], bf16, "ut")
            yt_sb = S([128, 4, 8], bf16, "yt")
            o_sb = S([128, 16], f32, "o_sb")

            for g in range(2):
                nc.gpsimd.memset(s_bd[g].bitcast(f32), 0.0)
                nc.gpsimd.memset(v_bd[g].bitcast(f32), 0.0)
                nc.gpsimd.memset(a1t_bd[g].bitcast(f32), 0.0)

            ps = {}

            # ---------- per-group level-1 stages (pairs a=2g, b=2g+1) -------
            def st_t1t_mm(g):
                t = ps_t1.tile([128, 4, 128], f32, tag="t1", name=f"t1_ps{g}")
                ps["t1", g] = t
                for h in range(2):
                    for c in range(2):
                        nc.tensor.matmul(t[:, 2 * h + c, :],
                                         x_ap(2 * g + h)[:, 128 * c:128 * (c + 1)],
                                         aT_ap(2 * g + h), start=True, stop=True)

            def st_t1t_ev(g):
                nc.vector.tensor_copy(out=t1t_sb[g][:, 0:256],
                                      in_=ps["t1", g][:, 0:2, :])
                nc.scalar.copy(out=t1t_sb[g][:, 256:512],
                               in_=ps["t1", g][:, 2:4, :])

            def _h_mm(g, h):
                p = 2 * g + h
                t = ps_hh.tile([128, 512], f32, tag="hh", name=f"hh_ps{p}")
                ps["hh", p] = t
                t1 = t1t_sb[g]
                for o in range(2):   # HpT first: feeds the softmax chain
                    for c in range(2):
                        nc.tensor.matmul(t[:, 256 + 128 * o:256 + 128 * (o + 1)],
                                         w1p(c, o),
                                         t1[:, 256 * h + 128 * c:256 * h + 128 * (c + 1)],
                                         start=(c == 0), stop=(c == 1))
                for c in range(2):
                    nc.tensor.matmul(t[:, 0:256],
                                     t1[:, 256 * h + 128 * c:256 * h + 128 * (c + 1)],
                                     w1e(c), start=(c == 0), stop=(c == 1))

            def st_h_mm_a(g):
                _h_mm(g, 0)

            def st_h_mm_b(g):
                _h_mm(g, 1)

            def _h_ev(g, h):
                p = 2 * g + h
                nc.scalar.activation(out=h_sb[p][:, 256:512],
                                     in_=ps["hh", p][:, 256:512], func=Relu)
                nc.vector.tensor_relu(out=h_sb[p][:, 0:256],
                                      in_=ps["hh", p][:, 0:256])

            def st_h_ev_a(g):
                _h_ev(g, 0)

            def st_h_ev_b(g):
                _h_ev(g, 1)

            def st_g_mm(g):
                t = ps_pl.tile([128, 4, 32], f32, tag="pl", name=f"gs_ps{g}")
                ps["gs", g] = t
                for h in range(2):
                    for c in range(2):
                        nc.tensor.matmul(t[:, h, :],
                                         h_sb[2 * g + h][:, 256 + 128 * c:256 + 128 * (c + 1)],
                                         w2p(c), start=(c == 0), stop=(c == 1))

            def st_g_ev(g):
                nc.vector.tensor_copy(out=g_sb[g], in_=ps["gs", g][:, 0:2, :])

            def st_slog_mm(g):
                for h in range(2):
                    nc.tensor.matmul(ps["gs", g][:, 2 + h, :], aT_ap(2 * g + h),
                                     g_sb[g][:, 32 * h:32 * h + 32],
                                     start=True, stop=True)

            def st_sm_exp(g):
                nc.scalar.activation(out=s_exp[g], in_=ps["gs", g][:, 2:4, :],
                                     func=Exp)

            def st_sm_sum(g):
                nc.vector.reduce_sum(out=sumexp[g], in_=s_exp[g], axis=AX)
                nc.vector.reciprocal(out=rsum[g], in_=sumexp[g])

            def st_sbd(g):
                for h in range(2):
                    nc.scalar.mul(out=s_bd[g][0:64, 64 * h:64 * h + 32],
                                  in_=s_exp[g][0:64, h, :],
                                  mul=rsum[g][0:64, h:h + 1])
                    nc.vector.tensor_scalar_mul(
                        out=s_bd[g][64:128, 64 * h + 32:64 * h + 64],
                        in0=s_exp[g][64:128, h, :],
                        scalar1=rsum[g][64:128, h:h + 1])

            def st_t2t_mm(g):
                t = ps_mid.tile([128, 4, 128], f32, tag="mid", name=f"t2t_ps{g}")
                ps["t2t", g] = t
                for h in range(2):
                    for c in range(2):
                        nc.tensor.matmul(t[:, 2 * h + c, :],
                                         h_sb[2 * g + h][:, 128 * c:128 * (c + 1)],
                                         aT_ap(2 * g + h), start=True, stop=True)

            def st_t2t_ev(g):
                nc.vector.tensor_copy(out=t2t_sb[g][:, 0:256],
                                      in_=ps["t2t", g][:, 0:2, :])
                nc.scalar.copy(out=t2t_sb[g][:, 256:512],
                               in_=ps["t2t", g][:, 2:4, :])

            def st_z_mm(g):
                t = ps_hh.tile([128, 512], f32, tag="hh", name=f"z_ps{g}")
                ps["z", g] = t
                for h in range(2):
                    for c in range(2):
                        nc.tensor.matmul(t[:, 256 * h:256 * (h + 1)],
                                         t2t_sb[g][:, 256 * h + 128 * c:256 * h + 128 * (c + 1)],
                                         w2e(c), start=(c == 0), stop=(c == 1))

            def st_z_ev(g):
                nc.vector.tensor_copy(out=z_sb[g][:, 0:256],
                                      in_=ps["z", g][:, 0:256])
                nc.scalar.copy(out=z_sb[g][:, 256:512],
                               in_=ps["z", g][:, 256:512])

            def st_x1q_mm(g):
                t = ps_pl.tile([128, 448], f32, tag="pl", name=f"pl_ps{g}")
                ps["pl", g] = t
                # identity seed for A1t + I (no data deps; runs early)
                for h in range(2):
                    nc.tensor.matmul(t[64 * h:64 * h + 64, 384:448],
                                     identb[0:64, :], identb[0:64, :],
                                     start=True, stop=False)
                for h in range(2):
                    nc.tensor.matmul(t[64 * h:64 * h + 64, 0:256],
                                     s_bd[g][:, 64 * h:64 * h + 64],
                                     z_sb[g][:, 256 * h:256 * (h + 1)],
                                     start=True, stop=True)
                for h in range(2):
                    nc.tensor.matmul(t[:, 256 + 64 * h:256 + 64 * (h + 1)],
                                     aT_ap(2 * g + h), s_bd[g][:, 64 * h:64 * h + 64],
                                     start=True, stop=True)

            def st_x1q_ev(g):
                nc.scalar.copy(out=x1_sb[g], in_=ps["pl", g][:, 0:256])
                nc.vector.tensor_sub(out=r_sb[g], in0=ps["pl", g][:, 256:384],
                                     in1=s_bd[g])

            def st_a1t_mm(g):
                for h in range(2):
                    nc.tensor.matmul(ps["pl", g][64 * h:64 * h + 64, 384:448],
                                     r_sb[g][:, 64 * h:64 * h + 64],
                                     s_bd[g][:, 64 * h:64 * h + 64],
                                     start=False, stop=True)

            def st_a1t_ev(g):
                nc.vector.reduce_sum(out=vred[g], in_=ps["pl", g][:, 384:448],
                                     axis=AX)
                nc.scalar.copy(out=a1t_bd[g][0:64, 0:64],
                               in_=ps["pl", g][0:64, 384:448])
                nc.gpsimd.tensor_copy(out=a1t_bd[g][64:128, 64:128],
                                      in_=ps["pl", g][64:128, 384:448])

            def st_vbd(g):
                for h in range(2):
                    sl0 = slice(64 * h, 64 * h + 32)
                    sl1 = slice(64 * h + 32, 64 * h + 64)
                    c0 = 4 * g + 2 * h
                    nc.scalar.copy(out=v_bd[g][sl0, c0:c0 + 1],
                                   in_=vred[g][sl0, :])
                    nc.vector.tensor_copy(out=v_bd[g][sl1, c0 + 1:c0 + 2],
                                          in_=vred[g][sl1, :])

            def st_tl2_mm(g):
                t = ps_t1.tile([128, 2, 128], f32, tag="t1", name=f"tl2_ps{g}")
                ps["tl2", g] = t
                for c in range(2):
                    nc.tensor.matmul(t[:, c, :],
                                     x1_sb[g][:, 128 * c:128 * (c + 1)],
                                     a1t_bd[g], start=True, stop=True)

            def st_tl2_ev(g):
                nc.vector.tensor_copy(out=tl2_sb[g][:, 0, :],
                                      in_=ps["tl2", g][:, 0, :])
                nc.scalar.copy(out=tl2_sb[g][:, 1, :],
                               in_=ps["tl2", g][:, 1, :])

            def st_h2_mm(g):
                t = ps_t1.tile([128, 256], f32, tag="t1", name=f"h2_ps{g}")
                ps["h2", g] = t
                for c in range(2):
                    nc.tensor.matmul(t, tl2_sb[g][:, c, :], w1e2(c),
                                     start=(c == 0), stop=(c == 1))

            def st_h2_ev(g):
                nc.scalar.activation(out=h2_sb[g][:, 0:128],
                                     in_=ps["h2", g][:, 0:128], func=Relu)
                nc.vector.tensor_relu(out=h2_sb[g][:, 128:256],
                                      in_=ps["h2", g][:, 128:256])

            def st_u_mm(g):
                if g == 0:
                    t = ps_mid.tile([128, 2, 8], f32, tag="mid", name="u_ps")
                    ps["u"] = t
                t = ps["u"]
                for c in range(2):
                    nc.tensor.matmul(t[:, c, :],
                                     h2_sb[g][:, 128 * c:128 * (c + 1)],
                                     v_bd[g], start=(g == 0), stop=(g == 1))

            STAGES = [st_t1t_mm, st_t1t_ev, st_h_mm_a, st_h_mm_b, st_h_ev_a,
                      st_h_ev_b, st_g_mm, st_g_ev, st_slog_mm, st_sm_exp,
                      st_sm_sum, st_sbd, st_t2t_mm, st_t2t_ev, st_z_mm,
                      st_z_ev, st_x1q_mm, st_x1q_ev, st_a1t_mm, st_a1t_ev,
                      st_vbd, st_tl2_mm, st_tl2_ev, st_h2_mm, st_h2_ev,
                      st_u_mm]
            NS = len(STAGES)

            def tail(_):
                nc.vector.tensor_copy(out=ut_sb, in_=ps["u"])
                y_ps = ps_mid.tile([128, 4, 8], f32, tag="mid", name="y_ps")
                for m in range(4):
                    for c in range(2):
                        nc.tensor.matmul(y_ps[:, m, :], wfold(c, m),
                                         ut_sb[:, c, :],
                                         start=(c == 0), stop=False)
                    nc.tensor.matmul(y_ps[:, m, :], b1row(m), ones8,
                                     start=False, stop=True)
                nc.scalar.activation(out=yt_sb, in_=y_ps, func=Relu)
                o_ps = ps_pl.tile([128, 16], f32, tag="pl", name="o_ps")
                for m in range(2):
                    for c in range(4):
                        nc.tensor.matmul(o_ps[:, 8 * m:8 * m + 8], lin2(c, m),
                                         yt_sb[:, c, :],
                                         start=(c == 0), stop=False)
                    nc.tensor.matmul(o_ps[:, 8 * m:8 * m + 8], b2row(m), ones8,
                                     start=False, stop=True)
                nc.scalar.copy(out=o_sb, in_=o_ps)
                nc.sync.dma_start(out=out_d[:, :], in_=o_sb)

            # ---------- wavefront emission over the two groups --------------
            import os
            lim = int(os.environ.get("KSTAGES", "999"))
            delay = [0, int(os.environ.get("KDELAY", "5"))]
            events = []
            for g in range(2):
                for s in range(min(NS, lim)):
                    events.append((delay[g] + s, g, STAGES[s], g))
            if lim > NS:
                events.append((delay[1] + NS, 9, tail, 0))
            events.sort(key=lambda e: (e[0], e[1]))
            ktrace = os.environ.get("KTRACE")
            for _, _, fn, arg in events:
                if ktrace:
                    blk = nc.m.functions[0].blocks[-1]
                    n0 = len(blk.instructions)
                fn(arg)
                if ktrace:
                    blk = nc.m.functions[0].blocks[-1]
                    names = [i.name for i in list(blk.instructions)[n0:]]
                    print(f"KTRACE {fn.__name__}(g={arg}): {names}")

    nc.finalize()
    return nc


def _prep_in_maps(inputs):
    import ml_dtypes
    bf = ml_dtypes.bfloat16
    f32 = np.float32
    x = np.ascontiguousarray(np.asarray(inputs["x"], f32))
    ei = np.asarray(inputs["edge_index"])
    src, dst = np.asarray(ei[0]), np.asarray(ei[1])

    A_blocks = np.zeros((B, NPG, NPG), f32)
    ok = (src // NPG) == (dst // NPG)
    A_blocks[src[ok] // NPG, src[ok] % NPG, dst[ok] % NPG] = 1.0
    I64 = np.eye(NPG, dtype=f32)

    W1p = np.asarray(inputs["pool0_W1"], f32)
    W2p = np.asarray(inputs["pool0_W2"], f32)
    W1e = np.asarray(inputs["emb0_W1"], f32)
    W2e = np.asarray(inputs["emb0_W2"], f32)
    W1e2 = np.asarray(inputs["emb1_W1"], f32)
    Wfold = (np.asarray(inputs["emb1_W2"], f32) @ np.asarray(inputs["lin1_W"], f32)) / 8.0
    L2 = np.asarray(inputs["lin2_W"], f32)
    b1 = np.asarray(inputs["lin1_b"], f32)
    b2 = np.asarray(inputs["lin2_b"], f32)

    dB1_np = np.hstack([np.hstack([W1p[128 * c:128 * (c + 1)], W1e[128 * c:128 * (c + 1)]])
                        for c in range(2)]).astype(bf)
    dB2_np = np.hstack([np.hstack([W2p[128 * c:128 * (c + 1)], W2e[128 * c:128 * (c + 1)]])
                        for c in range(2)]).astype(bf)
    brows = np.zeros((128, 768), f32)
    brows[0, 0:512] = b1
    brows[0, 512:768] = b2
    dD_np = np.hstack([W1e2[0:128], W1e2[128:256],
                       Wfold[0:128], Wfold[128:256],
                       L2[0:128], L2[128:256], L2[256:384], L2[384:512],
                       brows]).astype(bf)

    in_maps = []
    for core in range(NC_COUNT):
        halves = []
        for p in range(PAIRS):
            g0 = GPC * core + 2 * p
            xp = x[g0 * NPG:(g0 + 2) * NPG]                  # (128, 256)
            aT = np.zeros((128, 128), f32)
            aT[:64, :64] = (A_blocks[g0] + I64).T
            aT[64:, 64:] = (A_blocks[g0 + 1] + I64).T
            halves.append(np.hstack([xp, aT]))
        dA_np = np.hstack(halves[:2]).astype(bf)
        dC_np = np.hstack(halves[2:]).astype(bf)
        in_maps.append({"dA": np.ascontiguousarray(dA_np),
                        "dB1": np.ascontiguousarray(dB1_np),
                        "dB2": np.ascontiguousarray(dB2_np),
                        "dC": np.ascontiguousarray(dC_np),
                        "dD": np.ascontiguousarray(dD_np)})
    return in_maps


def kernel(**inputs) -> np.ndarray:
    global _BUILT
    from concourse.bass_utils import run_bass_kernel_spmd

    if _BUILT is None:
        _BUILT = _build()
    nc = _BUILT
    in_maps = _prep_in_maps(inputs)
    res = run_bass_kernel_spmd(nc, in_maps, core_ids=list(range(NC_COUNT)))
    out = np.zeros((B, 256), np.float32)
    for c in range(NC_COUNT):
        ot = np.asarray(res.results[c]["out_t"], np.float32)   # (128, 16)
        out[GPC * c:GPC * (c + 1)] = (
            ot.reshape(128, 2, 8).transpose(2, 1, 0).reshape(GPC, 256))
    return out


# revision 10
# speedup vs baseline: 1.0056x; 1.0056x over previous
"""Trainium2 Bass kernel for nn_DiffPoolEncoder (64 graphs x 64 nodes, D=256).

Data-parallel over graphs: 8 graphs per NeuronCore, packed as 4 pairs of two
64-node graphs per 128-partition tile. All matmul operands are bf16 (PSUM
accumulates fp32).

Math reformulations (validated against the jax reference in numpy):
  * Level-2 pooling branch is dead: softmax rows sum to 1, so
    mean_s(S2^T Z2) = colsum(Z2)/P2; colsum(Z2) = H2^T v with v = A1hat^T 1.
  * (1/8) and emb1_W2 @ lin1_W fold into one host-side constant Wfold.
  * Pool branch projects 256->32 BEFORE the second propagation:
    S_logits = Ahat (Hp W2p) instead of (Ahat Hp) W2p.
  * A1hat^T = S^T A^T S + I computed without a raw-A input:
    Q = Ahat S, R = Q - S = A S, A1t = R^T S; the +I is accumulated in PSUM
    via an identity matmul, so the row-sum v needs no +1 fixup.
  * Z = (Ahat He) W2e is built off the critical path via the transposed
    propagation T2t = He^T Ahat^T, so X1 = S^T Z is a single matmul stage
    after the softmax.
  * Softmax skips the max-subtraction (logits are bounded ~ +-55 on this
    distribution; exp stays far below f32/bf16 overflow).
  * lin1/lin2 biases fold into the matmul accumulations as rank-1 updates
    (bias row stationary x ones moving), so each linear needs just one
    PSUM->SBUF eviction and the final result is DMA'd straight from PSUM.
"""
import numpy as np

NC_COUNT = 8
B, NPG, D = 64, 64, 256
GPC = B // NC_COUNT     # graphs per core = 8
PAIRS = GPC // 2        # 4 pair-tiles of 128 nodes per core

# --- consolidated input column layouts (bf16 cols per partition) ---
# dA / dC: [x(256) | ahatT(128)] x 2 pairs             -> 768 cols
# dB1: c*512 + [W1p 0:256 | W1e 256:512]               -> 1024 cols
# dB2: c*288 + [w2p 0:32 | w2e 32:288]                 -> 576 cols
# dD: w1e2 c*256 (512), wfold 512 + c*512 + m*128 (1024),
#     lin2 1536 + c*256 + m*128 (1024),
#     bias rows: row 0, cols 2560+128*k for k in 0..3 -> b1 chunks,
#                row 0, cols 3072+128*k for k in 0..1 -> b2 chunks
DA_COLS = 768
DB1_COLS = 1024
DB2_COLS = 576
DC_COLS = 768
DD_COLS = 3328

_BUILT = None


def _build():
    import concourse.bacc as bacc
    import concourse.tile as tile
    from concourse import mybir

    f32 = mybir.dt.float32
    bf16 = mybir.dt.bfloat16
    nc = bacc.Bacc("TRN2", target_bir_lowering=False, debug=False,
                   num_devices=NC_COUNT)

    dA = nc.dram_tensor("dA", [128, DA_COLS], bf16, kind="ExternalInput")
    dB1 = nc.dram_tensor("dB1", [128, DB1_COLS], bf16, kind="ExternalInput")
    dB2 = nc.dram_tensor("dB2", [128, DB2_COLS], bf16, kind="ExternalInput")
    dC = nc.dram_tensor("dC", [128, DC_COLS], bf16, kind="ExternalInput")
    dD = nc.dram_tensor("dD", [128, DD_COLS], bf16, kind="ExternalInput")
    out_d = nc.dram_tensor("out_t", [128, 16], f32, kind="ExternalOutput")

    Relu = mybir.ActivationFunctionType.Relu
    Exp = mybir.ActivationFunctionType.Exp
    AX = mybir.AxisListType.X

    with tile.TileContext(nc) as tc:
        with (
            tc.tile_pool(name="singles", bufs=1) as sg,
            tc.tile_pool(name="ps_t1", bufs=2, space="PSUM") as ps_t1,
            tc.tile_pool(name="ps_hh", bufs=2, space="PSUM") as ps_hh,
            tc.tile_pool(name="ps_mid", bufs=2, space="PSUM") as ps_mid,
            tc.tile_pool(name="ps_pl", bufs=2, space="PSUM") as ps_pl,
        ):
            # ---------- input DMAs (issued back to back; HWDGE pipelines) ---
            dA_sb = sg.tile([128, DA_COLS], bf16, tag="dA", name="dA_sb")
            nc.sync.dma_start(out=dA_sb, in_=dA[:, :])
            dB1_sb = sg.tile([128, DB1_COLS], bf16, tag="dB1", name="dB1_sb")
            nc.sync.dma_start(out=dB1_sb, in_=dB1[:, :])
            dC_sb = sg.tile([128, DC_COLS], bf16, tag="dC", name="dC_sb")
            nc.sync.dma_start(out=dC_sb, in_=dC[:, :])
            dB2_sb = sg.tile([128, DB2_COLS], bf16, tag="dB2", name="dB2_sb")
            nc.sync.dma_start(out=dB2_sb, in_=dB2[:, :])
            dD_sb = sg.tile([128, DD_COLS], bf16, tag="dD", name="dD_sb")
            nc.sync.dma_start(out=dD_sb, in_=dD[:, :])

            def x_ap(p):
                t = dA_sb if p < 2 else dC_sb
                o = 384 * (p % 2)
                return t[:, o:o + 256]

            def aT_ap(p):
                t = dA_sb if p < 2 else dC_sb
                o = 384 * (p % 2) + 256
                return t[:, o:o + 128]

            def w1p(c, o):
                return dB1_sb[:, 512 * c + 128 * o:512 * c + 128 * (o + 1)]

            def w1e(c):
                return dB1_sb[:, 512 * c + 256:512 * c + 512]

            def w2p(c):
                return dB2_sb[:, 288 * c:288 * c + 32]

            def w2e(c):
                return dB2_sb[:, 288 * c + 32:288 * c + 288]

            def w1e2(c):
                return dD_sb[:, 256 * c:256 * (c + 1)]

            def wfold(c, m):
                o = 512 + 512 * c + 128 * m
                return dD_sb[:, o:o + 128]

            def lin2(c, m):
                o = 1536 + 256 * c + 128 * m
                return dD_sb[:, o:o + 128]

            def b1row(m):
                return dD_sb[0:1, 2560 + 128 * m:2560 + 128 * (m + 1)]

            def b2row(m):
                return dD_sb[0:1, 3072 + 128 * m:3072 + 128 * (m + 1)]

            # ---------- persistent SBUF tiles ------------------------------
            def S(shape, dt, nm):
                return sg.tile(shape, dt, tag=nm, name=nm)

            # trigger the Act engine's activation-table load at t=0 (its
            # 1.28us LoadActFuncSet otherwise lands behind the first
            # eviction's semaphore wait, stalling the whole Act queue)
            ones8 = S([1, 8], bf16, "ones8")
            nc.gpsimd.memset(ones8, 1.0)
            warm = S([1, 8], f32, "warm")
            nc.scalar.activation(out=warm, in_=ones8, func=Relu)

            ident = S([128, 64], f32, "ident")     # I64 stacked twice
            nc.gpsimd.memset(ident, 0.0)
            for h in (0, 1):
                nc.gpsimd.affine_select(
                    out=ident[64 * h:64 * h + 64, :],
                    in_=ident[64 * h:64 * h + 64, :],
                    compare_op=mybir.AluOpType.not_equal,
                    fill=1.0, base=0, pattern=[[-1, 64]], channel_multiplier=1)
            identb = S([128, 64], bf16, "identb")
            nc.vector.tensor_copy(out=identb, in_=ident)

            # one SBUF tile per writing engine: same-tile writes from two
            # engines serialize (tile-granular WAW tracking costs a ~220ns
            # cross-engine hop), so every split eviction gets its own tile
            t1t_sb = [[S([128, 256], bf16, f"t1t{g}_{h}") for h in range(2)]
                      for g in range(2)]
            he_sb = [S([128, 256], bf16, f"he{p}") for p in range(PAIRS)]
            hp_sb = [S([128, 256], bf16, f"hp{p}") for p in range(PAIRS)]
            g_sb = [S([128, 64], bf16, f"g{g}") for g in range(2)]
            s_exp = [S([128, 64], bf16, f"sexp{g}") for g in range(2)]
            sumexp = [S([128, 2], f32, f"sumexp{g}") for g in range(2)]
            rsum = [S([128, 2], f32, f"rsum{g}") for g in range(2)]
            s_bd = [S([128, 128], bf16, f"sbd{g}") for g in range(2)]
            t2t_sb = [[S([128, 256], bf16, f"t2t{g}_{h}") for h in range(2)]
                      for g in range(2)]
            z_sb = [[S([128, 256], bf16, f"z{g}_{h}") for h in range(2)]
                    for g in range(2)]
            x1_sb = [S([128, 256], bf16, f"x1{g}") for g in range(2)]
            r_sb = [S([128, 128], bf16, f"r{g}") for g in range(2)]
            a1t_bd = [S([128, 128], bf16, f"a1t{g}") for g in range(2)]
            vred = [S([128, 1], f32, f"vred{g}") for g in range(2)]
            v_bd = [S([128, 8], bf16, f"vbd{g}") for g in range(2)]
            tl2_sb = [[S([128, 128], bf16, f"tl2_{g}_{c}") for c in range(2)]
                      for g in range(2)]
            h2_sb = [[S([128, 128], bf16, f"h2_{g}_{c}") for c in range(2)]
                     for g in range(2)]
            ut_sb = S([128, 2, 8], bf16, "ut")
            yt_sb = S([128, 4, 8], bf16, "yt")
            o_sb = S([128, 16], f32, "o_sb")

            for g in range(2):
                nc.gpsimd.memset(s_bd[g].bitcast(f32), 0.0)
                nc.gpsimd.memset(v_bd[g].bitcast(f32), 0.0)
                nc.gpsimd.memset(a1t_bd[g].bitcast(f32), 0.0)

            ps = {}

            # ---------- per-group level-1 stages (pairs a=2g, b=2g+1) -------
            def st_t1t_mm(g):
                t = ps_t1.tile([128, 4, 128], f32, tag="t1", name=f"t1_ps{g}")
                ps["t1", g] = t
                for h in range(2):
                    for c in range(2):
                        nc.tensor.matmul(t[:, 2 * h + c, :],
                                         x_ap(2 * g + h)[:, 128 * c:128 * (c + 1)],
                                         aT_ap(2 * g + h), start=True, stop=True)

            def st_t1t_ev(g):
                nc.vector.tensor_copy(out=t1t_sb[g][0],
                                      in_=ps["t1", g][:, 0:2, :])
                nc.scalar.copy(out=t1t_sb[g][1],
                               in_=ps["t1", g][:, 2:4, :])

            def _h_mm(g, h):
                p = 2 * g + h
                t = ps_hh.tile([128, 512], f32, tag="hh", name=f"hh_ps{p}")
                ps["hh", p] = t
                t1 = t1t_sb[g][h]
                for o in range(2):   # HpT first: feeds the softmax chain
                    for c in range(2):
                        nc.tensor.matmul(t[:, 256 + 128 * o:256 + 128 * (o + 1)],
                                         w1p(c, o),
                                         t1[:, 128 * c:128 * (c + 1)],
                                         start=(c == 0), stop=(c == 1))
                for c in range(2):
                    nc.tensor.matmul(t[:, 0:256],
                                     t1[:, 128 * c:128 * (c + 1)],
                                     w1e(c), start=(c == 0), stop=(c == 1))

            def st_h_mm_a(g):
                _h_mm(g, 0)

            def st_h_mm_b(g):
                _h_mm(g, 1)

            def _h_ev(g, h):
                p = 2 * g + h
                nc.scalar.activation(out=hp_sb[p],
                                     in_=ps["hh", p][:, 256:512], func=Relu)
                nc.vector.tensor_relu(out=he_sb[p],
                                      in_=ps["hh", p][:, 0:256])

            def st_h_ev_a(g):
                _h_ev(g, 0)

            def st_h_ev_b(g):
                _h_ev(g, 1)

            # pl psum tile [128, 448] is phased: cols 0:64 hold g = Hp W2p and
            # 64:128 the softmax logits early on; x1 later overwrites 0:256
            # (a true dependency: x1 needs the softmax of those logits),
            # Q sits at 256:384 and A1t + I at 384:448.
            def st_g_mm(g):
                t = ps_pl.tile([128, 448], f32, tag="pl", name=f"pl_ps{g}")
                ps["pl", g] = t
                for h in range(2):
                    for c in range(2):
                        nc.tensor.matmul(t[:, 32 * h:32 * h + 32],
                                         hp_sb[2 * g + h][:, 128 * c:128 * (c + 1)],
                                         w2p(c), start=(c == 0), stop=(c == 1))
                # identity seed for A1t + I (no data deps; runs early)
                for h in range(2):
                    nc.tensor.matmul(t[64 * h:64 * h + 64, 384:448],
                                     identb[0:64, :], identb[0:64, :],
                                     start=True, stop=False)

            def st_g_ev(g):
                nc.vector.tensor_copy(out=g_sb[g], in_=ps["pl", g][:, 0:64])

            def st_slog_mm(g):
                for h in range(2):
                    nc.tensor.matmul(ps["pl", g][:, 64 + 32 * h:96 + 32 * h],
                                     aT_ap(2 * g + h),
                                     g_sb[g][:, 32 * h:32 * h + 32],
                                     start=True, stop=True)

            def st_sm_exp(g):
                nc.scalar.activation(out=s_exp[g], in_=ps["pl", g][:, 64:128],
                                     func=Exp)

            def st_sm_sum(g):
                for h in range(2):
                    nc.vector.reduce_sum(out=sumexp[g][:, h:h + 1],
                                         in_=s_exp[g][:, 32 * h:32 * h + 32],
                                         axis=AX)
                nc.vector.reciprocal(out=rsum[g], in_=sumexp[g])

            def st_sbd(g):
                for h in range(2):
                    nc.vector.tensor_scalar_mul(
                        out=s_bd[g][0:64, 64 * h:64 * h + 32],
                        in0=s_exp[g][0:64, 32 * h:32 * h + 32],
                        scalar1=rsum[g][0:64, h:h + 1])
                    nc.vector.tensor_scalar_mul(
                        out=s_bd[g][64:128, 64 * h + 32:64 * h + 64],
                        in0=s_exp[g][64:128, 32 * h:32 * h + 32],
                        scalar1=rsum[g][64:128, h:h + 1])

            def st_t2t_mm(g):
                t = ps_mid.tile([128, 4, 128], f32, tag="mid", name=f"t2t_ps{g}")
                ps["t2t", g] = t
                for h in range(2):
                    for c in range(2):
                        nc.tensor.matmul(t[:, 2 * h + c, :],
                                         he_sb[2 * g + h][:, 128 * c:128 * (c + 1)],
                                         aT_ap(2 * g + h), start=True, stop=True)

            def st_t2t_ev(g):
                nc.gpsimd.tensor_copy(out=t2t_sb[g][0],
                                      in_=ps["t2t", g][:, 0:2, :])
                nc.scalar.copy(out=t2t_sb[g][1],
                               in_=ps["t2t", g][:, 2:4, :])

            def st_z_mm(g):
                t = ps_hh.tile([128, 512], f32, tag="hh", name=f"z_ps{g}")
                ps["z", g] = t
                for h in range(2):
                    for c in range(2):
                        nc.tensor.matmul(t[:, 256 * h:256 * (h + 1)],
                                         t2t_sb[g][h][:, 128 * c:128 * (c + 1)],
                                         w2e(c), start=(c == 0), stop=(c == 1))

            def st_z_ev(g):
                nc.vector.tensor_copy(out=z_sb[g][0], in_=ps["z", g][:, 0:256])
                nc.scalar.copy(out=z_sb[g][1], in_=ps["z", g][:, 256:512])

            def st_x1q_mm(g):
                t = ps["pl", g]
                for h in range(2):
                    nc.tensor.matmul(t[64 * h:64 * h + 64, 0:256],
                                     s_bd[g][:, 64 * h:64 * h + 64],
                                     z_sb[g][h], start=True, stop=True)
                for h in range(2):
                    nc.tensor.matmul(t[:, 256 + 64 * h:256 + 64 * (h + 1)],
                                     aT_ap(2 * g + h), s_bd[g][:, 64 * h:64 * h + 64],
                                     start=True, stop=True)

            def st_x1q_ev(g):
                nc.scalar.copy(out=x1_sb[g], in_=ps["pl", g][:, 0:256])
                nc.vector.tensor_sub(out=r_sb[g], in0=ps["pl", g][:, 256:384],
                                     in1=s_bd[g])

            def st_a1t_mm(g):
                for h in range(2):
                    nc.tensor.matmul(ps["pl", g][64 * h:64 * h + 64, 384:448],
                                     r_sb[g][:, 64 * h:64 * h + 64],
                                     s_bd[g][:, 64 * h:64 * h + 64],
                                     start=False, stop=True)

            def st_a1t_ev(g):
                nc.vector.reduce_sum(out=vred[g], in_=ps["pl", g][:, 384:448],
                                     axis=AX)
                nc.gpsimd.tensor_copy(out=a1t_bd[g][0:64, 0:64],
                                      in_=ps["pl", g][0:64, 384:448])
                nc.gpsimd.tensor_copy(out=a1t_bd[g][64:128, 64:128],
                                      in_=ps["pl", g][64:128, 384:448])

            def st_vbd(g):
                for h in range(2):
                    sl0 = slice(64 * h, 64 * h + 32)
                    sl1 = slice(64 * h + 32, 64 * h + 64)
                    c0 = 4 * g + 2 * h
                    nc.vector.tensor_copy(out=v_bd[g][sl0, c0:c0 + 1],
                                          in_=vred[g][sl0, :])
                    nc.vector.tensor_copy(out=v_bd[g][sl1, c0 + 1:c0 + 2],
                                          in_=vred[g][sl1, :])

            def st_tl2_mm(g):
                t = ps_t1.tile([128, 4, 128], f32, tag="t1", name=f"tl2_ps{g}")
                ps["tl2", g] = t
                for c in range(2):
                    nc.tensor.matmul(t[:, c, :],
                                     x1_sb[g][:, 128 * c:128 * (c + 1)],
                                     a1t_bd[g], start=True, stop=True)

            def st_tl2_ev(g):
                nc.vector.tensor_copy(out=tl2_sb[g][0], in_=ps["tl2", g][:, 0, :])
                nc.scalar.copy(out=tl2_sb[g][1], in_=ps["tl2", g][:, 1, :])

            def st_h2_mm(g):
                # h2 shares the tl2 psum tile (cols 256:512); no bank churn
                t = ps["tl2", g]
                for c in range(2):
                    nc.tensor.matmul(t[:, 2:4, :], tl2_sb[g][c], w1e2(c),
                                     start=(c == 0), stop=(c == 1))

            def st_h2_ev(g):
                nc.scalar.activation(out=h2_sb[g][0],
                                     in_=ps["tl2", g][:, 2, :], func=Relu)
                nc.vector.tensor_relu(out=h2_sb[g][1],
                                      in_=ps["tl2", g][:, 3, :])

            def st_u_mm(g):
                if g == 0:
                    t = ps_mid.tile([128, 2, 8], f32, tag="mid", name="u_ps")
                    ps["u"] = t
                t = ps["u"]
                for c in range(2):
                    nc.tensor.matmul(t[:, c, :], h2_sb[g][c],
                                     v_bd[g], start=(g == 0), stop=(g == 1))

            STAGES = [st_t1t_mm, st_t1t_ev, st_h_mm_a, st_h_mm_b, st_h_ev_a,
                      st_h_ev_b, st_g_mm, st_g_ev, st_slog_mm, st_sm_exp,
                      st_sm_sum, st_sbd, st_t2t_mm, st_t2t_ev, st_z_mm,
                      st_z_ev, st_x1q_mm, st_x1q_ev, st_a1t_mm, st_a1t_ev,
                      st_vbd, st_tl2_mm, st_tl2_ev, st_h2_mm, st_h2_ev,
                      st_u_mm]
            NS = len(STAGES)

            def tail(_):
                nc.vector.tensor_copy(out=ut_sb, in_=ps["u"])
                y_ps = ps_mid.tile([128, 4, 8], f32, tag="mid", name="y_ps")
                for m in range(4):
                    for c in range(2):
                        nc.tensor.matmul(y_ps[:, m, :], wfold(c, m),
                                         ut_sb[:, c, :],
                                         start=(c == 0), stop=False)
                    nc.tensor.matmul(y_ps[:, m, :], b1row(m), ones8,
                                     start=False, stop=True)
                nc.scalar.activation(out=yt_sb, in_=y_ps, func=Relu)
                o_ps = ps_pl.tile([128, 16], f32, tag="pl", name="o_ps")
                for m in range(2):
                    for c in range(4):
                        nc.tensor.matmul(o_ps[:, 8 * m:8 * m + 8], lin2(c, m),
                                         yt_sb[:, c, :],
                                         start=(c == 0), stop=False)
                    nc.tensor.matmul(o_ps[:, 8 * m:8 * m + 8], b2row(m), ones8,
                                     start=False, stop=True)
                nc.scalar.copy(out=o_sb, in_=o_ps)
                nc.sync.dma_start(out=out_d[:, :], in_=o_sb)

            # ---------- wavefront emission over the two groups --------------
            import os
            lim = int(os.environ.get("KSTAGES", "999"))
            delay = [0, int(os.environ.get("KDELAY", "5"))]
            events = []
            for g in range(2):
                for s in range(min(NS, lim)):
                    events.append((delay[g] + s, g, STAGES[s], g))
            if lim > NS:
                events.append((delay[1] + NS, 9, tail, 0))
            events.sort(key=lambda e: (e[0], e[1]))
            ktrace = os.environ.get("KTRACE")
            for _, _, fn, arg in events:
                if ktrace:
                    blk = nc.m.functions[0].blocks[-1]
                    n0 = len(blk.instructions)
                fn(arg)
                if ktrace:
                    blk = nc.m.functions[0].blocks[-1]
                    names = [i.name for i in list(blk.instructions)[n0:]]
                    print(f"KTRACE {fn.__name__}(g={arg}): {names}")

    nc.finalize()
    return nc


def _prep_in_maps(inputs):
    import ml_dtypes
    bf = ml_dtypes.bfloat16
    f32 = np.float32
    x = np.ascontiguousarray(np.asarray(inputs["x"], f32))
    ei = np.asarray(inputs["edge_index"])
    src, dst = np.asarray(ei[0]), np.asarray(ei[1])

    A_blocks = np.zeros((B, NPG, NPG), f32)
    ok = (src // NPG) == (dst // NPG)
    A_blocks[src[ok] // NPG, src[ok] % NPG, dst[ok] % NPG] = 1.0
    I64 = np.eye(NPG, dtype=f32)

    W1p = np.asarray(inputs["pool0_W1"], f32)
    W2p = np.asarray(inputs["pool0_W2"], f32)
    W1e = np.asarray(inputs["emb0_W1"], f32)
    W2e = np.asarray(inputs["emb0_W2"], f32)
    W1e2 = np.asarray(inputs["emb1_W1"], f32)
    Wfold = (np.asarray(inputs["emb1_W2"], f32) @ np.asarray(inputs["lin1_W"], f32)) / 8.0
    L2 = np.asarray(inputs["lin2_W"], f32)
    b1 = np.asarray(inputs["lin1_b"], f32)
    b2 = np.asarray(inputs["lin2_b"], f32)

    dB1_np = np.hstack([np.hstack([W1p[128 * c:128 * (c + 1)], W1e[128 * c:128 * (c + 1)]])
                        for c in range(2)]).astype(bf)
    dB2_np = np.hstack([np.hstack([W2p[128 * c:128 * (c + 1)], W2e[128 * c:128 * (c + 1)]])
                        for c in range(2)]).astype(bf)
    brows = np.zeros((128, 768), f32)
    brows[0, 0:512] = b1
    brows[0, 512:768] = b2
    dD_np = np.hstack([W1e2[0:128], W1e2[128:256],
                       Wfold[0:128], Wfold[128:256],
                       L2[0:128], L2[128:256], L2[256:384], L2[384:512],
                       brows]).astype(bf)

    in_maps = []
    for core in range(NC_COUNT):
        halves = []
        for p in range(PAIRS):
            g0 = GPC * core + 2 * p
            xp = x[g0 * NPG:(g0 + 2) * NPG]                  # (128, 256)
            aT = np.zeros((128, 128), f32)
            aT[:64, :64] = (A_blocks[g0] + I64).T
            aT[64:, 64:] = (A_blocks[g0 + 1] + I64).T
            halves.append(np.hstack([xp, aT]))
        dA_np = np.hstack(halves[:2]).astype(bf)
        dC_np = np.hstack(halves[2:]).astype(bf)
        in_maps.append({"dA": np.ascontiguousarray(dA_np),
                        "dB1": np.ascontiguousarray(dB1_np),
                        "dB2": np.ascontiguousarray(dB2_np),
                        "dC": np.ascontiguousarray(dC_np),
                        "dD": np.ascontiguousarray(dD_np)})
    return in_maps


def kernel(**inputs) -> np.ndarray:
    global _BUILT
    from concourse.bass_utils import run_bass_kernel_spmd

    if _BUILT is None:
        _BUILT = _build()
    nc = _BUILT
    in_maps = _prep_in_maps(inputs)
    res = run_bass_kernel_spmd(nc, in_maps, core_ids=list(range(NC_COUNT)))
    out = np.zeros((B, 256), np.float32)
    for c in range(NC_COUNT):
        ot = np.asarray(res.results[c]["out_t"], np.float32)   # (128, 16)
        out[GPC * c:GPC * (c + 1)] = (
            ot.reshape(128, 2, 8).transpose(2, 1, 0).reshape(GPC, 256))
    return out
